# revision 7
# baseline (speedup 1.0000x reference)
"""Permutation scatter: out[perm[i]] = inputs[i]  (B=131072, D=512, f32).

Since perm is a permutation, out[j] = inputs[inv_perm[j]] -- a pure row
gather.  Strategy: shard the OUTPUT rows across the 8 cores and replicate
the full input to every core.  Core k owns output rows [k*R, (k+1)*R) and
gathers its 16384 rows (2 KiB each) from its local replica with indirect
DMAs, then writes its output shard contiguously.  No collectives; per-core
HBM traffic is the minimum possible (32 MiB read + 32 MiB write).  The
host only computes the inverse permutation (index math); all payload
movement happens on-device.

HW contract for indirect DMA (probed extensively): one op = one int32
index per partition (offset AP strictly [128, 1] -- [1,128]/[16,8] layouts
and DRAM dests crash the device; extra offset columns are ignored; multi-
run dest APs emit garbage).  Each op costs ~994ns fixed SWDGE ucode time
+ ~0.34ns/desc + ~300ns dispatch gap => ~1.4us per 128 rows, which lands
within 2% of the per-core HBM cost of those rows (2x2KiB / ~370GB/s).
The kernel is therefore simultaneously emission- and HBM-bound; the
remaining levers are startup latency (warm the Q7 ucode + gather ring
during the index-load flight) and store burst size (group=2).
"""

import numpy as np

B = 131072
D = 512
N_CORES = 8
R = B // N_CORES  # 16384 output rows per core
P = 128
NCH = R // P  # 128 chunks per core

SLOTS = 24  # rotating [128, GROUP*D] SBUF tiles
GROUP = 2  # 128-row gathers per store (4KiB/partition write bursts)
HEAD = 2  # chunks in the first index load (gates gather 0); 1 would be a
# non-contiguous 4B/partition load, so 2 is the minimum contiguous slice
WARM = True  # dummy indirect gather during idx flight warms Q7 + ring

_cached = None


def _indirect_dma(eng, out, in_, offset_col):
    """Mirror of bass.py indirect_dma_start (gather form).  Inlined so the
    emission stays probe-validated even if bass defaults change."""
    import concourse.mybir as mybir

    assert isinstance(in_.offset, int) and in_.offset == 0
    out_ap = eng.lower_ap_dma(out, for_indirect_dma=True)
    in_ap = eng.lower_ap_dma(in_, for_indirect_dma=True)
    offset_ap = eng.lower_ap_dma(offset_col)[0]
    in_ap.append(offset_ap)

    coef = 1
    for i in range(1, len(in_.shape)):
        coef *= in_.shape[i]
    in_ap[0].dynamic_ap_info = mybir.DynamicAccessPatternInfo(
        c=0,
        actual_ap=out.ap,
        indirect_dim_max_index=in_.shape[0],
        offset_expr=[
            mybir.DynamicAccessPatternOffsetExpr(
                coef=coef,
                aff_expr=mybir.DynamicAccessPatternOffsetExprAffExpr(
                    kind="IndirectArgId", arg_id=1
                ),
            )
        ],
    )
    return eng.add_instruction(
        mybir.InstDMACopy(
            name=eng.bass.get_next_instruction_name(),
            queue="qPoolDynamic",
            mode="Copy",
            ins=in_ap,
            outs=out_ap,
            oob_is_err=True,
            cce_op=mybir.AluOpType.bypass,
            single_packet=True,
        )
    )


def _build_nc(slots=SLOTS, group=GROUP, head=HEAD, warm=WARM):
    """Raw-Bass kernel: hand-rolled semaphores, minimal prologue/epilogue.
    ``group`` 128-row gathers land in one [128, group*D] SBUF tile written
    back with a single store (bigger write bursts).  ``slots`` tiles rotate.
    """
    from contextlib import ExitStack

    import concourse.bass as bass
    import concourse.mybir as mybir

    n_groups = NCH // group
    assert NCH % group == 0 and head % group == 0

    nc = bass.Bass(
        "TRN2",
        target_bir_lowering=False,
        debug=False,
        num_devices=N_CORES,
    )

    x = nc.dram_tensor("x", [B, D], mybir.dt.float32, kind="ExternalInput")
    idxT = nc.dram_tensor("idxT", [P, NCH], mybir.dt.int32, kind="ExternalInput")
    y = nc.dram_tensor("y", [R, D], mybir.dt.float32, kind="ExternalOutput")
    # Store target for group j: output rows [j*group*128, (j+1)*group*128),
    # with partition p holding the `group` CONSECUTIVE rows
    # [j*group*128 + p*group, j*group*128 + (p+1)*group) -- so each partition
    # writes one contiguous group*D*4-byte run (big store bursts).  Gather g
    # of the group fills tile columns [g*D, (g+1)*D), so its 128 indices are
    # idxT[:, j*group + g] = inv_k[j*group*128 + p*group + g] (_make_in_maps).
    y_r = y[:].rearrange("(j p g) d -> j p (g d)", p=P, g=group)

    with ExitStack() as ctx:
        it = ctx.enter_context(nc.sbuf_tensor("it", [P, NCH], mybir.dt.int32))
        wt = ctx.enter_context(nc.sbuf_tensor("wt", [P, D], mybir.dt.float32))
        wi = ctx.enter_context(nc.sbuf_tensor("wi", [P, 1], mybir.dt.int32))
        dts = [
            ctx.enter_context(
                nc.sbuf_tensor(f"dt{i}", [P, group * D], mybir.dt.float32)
            )
            for i in range(slots)
        ]
        # Per-slot semaphores with exact thresholds (a single cumulative sem
        # is racy: completions from the 16 SDMA engines interleave across
        # successive DMAs).  A slot's store waits for all `group` gathers of
        # its round (full sum = race-free); the next round's gathers wait for
        # that store.
        isem = nc.alloc_semaphore("isem")
        isem2 = nc.alloc_semaphore("isem2")
        wsem = nc.alloc_semaphore("wsem")
        gsems = [nc.alloc_semaphore(f"gsem{i}") for i in range(slots)]
        ssems = [nc.alloc_semaphore(f"ssem{i}") for i in range(slots)]

        def rounds(slot):  # number of groups handled by this slot
            return (n_groups - slot + slots - 1) // slots

        with nc.Block(no_gpsimd_drain=True) as block:

            @block.sync
            def _(sync):
                sync.dma_start(out=it[:, :head], in_=idxT[:, :head]).then_inc(
                    isem, 16
                )
                sync.dma_start(out=it[:, head:], in_=idxT[:, head:]).then_inc(
                    isem2, 16
                )
                for j in range(n_groups):
                    i, k = j % slots, j // slots
                    sync.wait_ge(gsems[i], (k + 1) * group * 16)
                    sync.dma_start(out=y_r[j], in_=dts[i][:]).then_inc(
                        ssems[i], 16
                    )
                for i in range(slots):
                    sync.wait_ge(ssems[i], rounds(i) * 16)
                sync.wait_ge(isem, 16)
                sync.wait_ge(isem2, 16)

            @block.gpsimd
            def _(g_):
                if warm:
                    # Warm the Q7 indirect ucode + the qGpSimdDynamic ring
                    # while the index head load is in flight: gather row 0
                    # into a scratch tile nothing else touches.
                    g_.memset(wi[:], 0)
                    _indirect_dma(g_, wt[:], x[:], wi[:]).then_inc(wsem, 16)
                g_.wait_ge(isem, 16)
                for j in range(n_groups):
                    i, k = j % slots, j // slots
                    if j * group == head:
                        g_.wait_ge(isem2, 16)
                    if j >= slots:
                        g_.wait_ge(ssems[i], k * 16)
                    for g in range(group):
                        c = j * group + g
                        _indirect_dma(
                            g_,
                            dts[i][:, g * D : (g + 1) * D],
                            x[:],
                            it[:, c : c + 1],
                        ).then_inc(gsems[i], 16)
                if warm:
                    g_.wait_ge(wsem, 16)

        # Block exit emitted per-engine drains + a sem-only barrier; all DMA
        # completions were explicitly waited above, so a plain range-clear
        # (no dge drain) suffices to make the NEFF re-executable.
        sem_nums = sorted(
            [isem.num, isem2.num, wsem.num]
            + [s.num for s in gsems]
            + [s.num for s in ssems]
        )
        assert sem_nums == list(range(sem_nums[0], sem_nums[-1] + 1))
        nc.gpsimd.sem_clear(range(sem_nums[0], sem_nums[-1] + 1))

    return nc


def _get_nc():
    global _cached
    if _cached is None:
        _cached = _build_nc()
    return _cached


def _make_in_maps(inputs, perm):
    x = np.ascontiguousarray(np.asarray(inputs, dtype=np.float32))
    p = np.asarray(perm).astype(np.int64)
    inv = np.empty(B, dtype=np.int32)
    inv[p] = np.arange(B, dtype=np.int32)
    maps = []
    for k in range(N_CORES):
        sl = inv[k * R : (k + 1) * R]
        # idxT[p, j*group + g] = inv_k[j*group*128 + p*group + g]
        n_groups = NCH // GROUP
        idxT = (
            sl.reshape(n_groups, P, GROUP).transpose(1, 0, 2).reshape(P, NCH)
        )
        maps.append({"x": x, "idxT": np.ascontiguousarray(idxT)})
    return maps


def kernel(**kw):
    from concourse.bass_utils import run_bass_kernel_spmd

    nc = _get_nc()
    in_maps = _make_in_maps(kw["inputs"], kw["perm"])
    res = run_bass_kernel_spmd(nc, in_maps, core_ids=list(range(N_CORES)))
    return np.concatenate([res.results[k]["y"] for k in range(N_CORES)], axis=0)


def run_traced(inputs, perm, **trace_kw):
    """test.py helper: same as kernel() but returns (out, BassKernelResults)."""
    from concourse.bass_utils import run_bass_kernel_spmd

    nc = _get_nc()
    in_maps = _make_in_maps(inputs, perm)
    res = run_bass_kernel_spmd(
        nc, in_maps, core_ids=list(range(N_CORES)), trace=True, **trace_kw
    )
    out = np.concatenate([res.results[k]["y"] for k in range(N_CORES)], axis=0)
    return out, res


# revision 8
# speedup vs baseline: 1.0116x; 1.0116x over previous
"""Permutation scatter: out[perm[i]] = inputs[i]  (B=131072, D=512, f32).

Since perm is a permutation, out[j] = inputs[inv_perm[j]] -- a pure row
gather.  Strategy: shard the OUTPUT rows across the 8 cores and replicate
the full input to every core.  Core k owns output rows [k*R, (k+1)*R) and
gathers its 16384 rows (2 KiB each) from its local replica with indirect
DMAs, then writes its output shard contiguously.  No collectives; per-core
HBM traffic is the minimum possible (32 MiB read + 32 MiB write).  The
host only computes the inverse permutation (index math); all payload
movement happens on-device.

HW contract for indirect DMA (probed extensively): one op = one int32
index per partition (offset AP strictly [128, 1] -- [1,128]/[16,8] layouts
and DRAM dests crash the device; extra offset columns are ignored; multi-
run dest APs emit garbage).  Each op costs ~994ns fixed SWDGE ucode time
+ ~0.34ns/desc + ~300ns dispatch gap => ~1.4us per 128 rows, which lands
within 2% of the per-core HBM cost of those rows (2x2KiB / ~370GB/s).
The kernel is therefore simultaneously emission- and HBM-bound; the
remaining levers are startup latency (warm the Q7 ucode + gather ring
during the index-load flight) and store burst size (group=2).
"""

import numpy as np

B = 131072
D = 512
N_CORES = 8
R = B // N_CORES  # 16384 output rows per core
P = 128
NCH = R // P  # 128 chunks per core

SLOTS = 24  # rotating [128, GROUP*D] SBUF tiles
GROUP = 1  # 128-row gathers per store (group=2 measured +2us: bigger final
# store chain and deeper gather->store latency, no mid-run HBM gain)
HEAD = 2  # chunks in the first index load (gates gather 0); 1 would be a
# non-contiguous 4B/partition load, so 2 is the minimum contiguous slice
WARM = True  # dummy indirect gather during idx flight warms Q7 + ring

_cached = None


def _indirect_dma(eng, out, in_, offset_col):
    """Mirror of bass.py indirect_dma_start (gather form).  Inlined so the
    emission stays probe-validated even if bass defaults change."""
    import concourse.mybir as mybir

    assert isinstance(in_.offset, int) and in_.offset == 0
    out_ap = eng.lower_ap_dma(out, for_indirect_dma=True)
    in_ap = eng.lower_ap_dma(in_, for_indirect_dma=True)
    offset_ap = eng.lower_ap_dma(offset_col)[0]
    in_ap.append(offset_ap)

    coef = 1
    for i in range(1, len(in_.shape)):
        coef *= in_.shape[i]
    in_ap[0].dynamic_ap_info = mybir.DynamicAccessPatternInfo(
        c=0,
        actual_ap=out.ap,
        indirect_dim_max_index=in_.shape[0],
        offset_expr=[
            mybir.DynamicAccessPatternOffsetExpr(
                coef=coef,
                aff_expr=mybir.DynamicAccessPatternOffsetExprAffExpr(
                    kind="IndirectArgId", arg_id=1
                ),
            )
        ],
    )
    return eng.add_instruction(
        mybir.InstDMACopy(
            name=eng.bass.get_next_instruction_name(),
            queue="qPoolDynamic",
            mode="Copy",
            ins=in_ap,
            outs=out_ap,
            oob_is_err=True,
            cce_op=mybir.AluOpType.bypass,
            single_packet=True,
        )
    )


def _build_nc(slots=SLOTS, group=GROUP, head=HEAD, warm=WARM):
    """Raw-Bass kernel: hand-rolled semaphores, minimal prologue/epilogue.
    ``group`` 128-row gathers land in one [128, group*D] SBUF tile written
    back with a single store (bigger write bursts).  ``slots`` tiles rotate.
    """
    from contextlib import ExitStack

    import concourse.bass as bass
    import concourse.mybir as mybir

    n_groups = NCH // group
    assert NCH % group == 0 and head % group == 0

    nc = bass.Bass(
        "TRN2",
        target_bir_lowering=False,
        debug=False,
        num_devices=N_CORES,
    )

    x = nc.dram_tensor("x", [B, D], mybir.dt.float32, kind="ExternalInput")
    idxT = nc.dram_tensor("idxT", [P, NCH], mybir.dt.int32, kind="ExternalInput")
    y = nc.dram_tensor("y", [R, D], mybir.dt.float32, kind="ExternalOutput")
    # Store target for group j: output rows [j*group*128, (j+1)*group*128),
    # with partition p holding the `group` CONSECUTIVE rows
    # [j*group*128 + p*group, j*group*128 + (p+1)*group) -- so each partition
    # writes one contiguous group*D*4-byte run (big store bursts).  Gather g
    # of the group fills tile columns [g*D, (g+1)*D), so its 128 indices are
    # idxT[:, j*group + g] = inv_k[j*group*128 + p*group + g] (_make_in_maps).
    y_r = y[:].rearrange("(j p g) d -> j p (g d)", p=P, g=group)

    with ExitStack() as ctx:
        it = ctx.enter_context(nc.sbuf_tensor("it", [P, NCH], mybir.dt.int32))
        wt = ctx.enter_context(nc.sbuf_tensor("wt", [P, D], mybir.dt.float32))
        wi = ctx.enter_context(nc.sbuf_tensor("wi", [P, 1], mybir.dt.int32))
        dts = [
            ctx.enter_context(
                nc.sbuf_tensor(f"dt{i}", [P, group * D], mybir.dt.float32)
            )
            for i in range(slots)
        ]
        # Per-slot semaphores with exact thresholds (a single cumulative sem
        # is racy: completions from the 16 SDMA engines interleave across
        # successive DMAs).  A slot's store waits for all `group` gathers of
        # its round (full sum = race-free); the next round's gathers wait for
        # that store.
        isem = nc.alloc_semaphore("isem")
        isem2 = nc.alloc_semaphore("isem2")
        wsem = nc.alloc_semaphore("wsem")
        gsems = [nc.alloc_semaphore(f"gsem{i}") for i in range(slots)]
        ssems = [nc.alloc_semaphore(f"ssem{i}") for i in range(slots)]

        def rounds(slot):  # number of groups handled by this slot
            return (n_groups - slot + slots - 1) // slots

        with nc.Block(no_gpsimd_drain=True) as block:

            @block.sync
            def _(sync):
                sync.dma_start(out=it[:, :head], in_=idxT[:, :head]).then_inc(
                    isem, 16
                )
                sync.dma_start(out=it[:, head:], in_=idxT[:, head:]).then_inc(
                    isem2, 16
                )
                for j in range(n_groups):
                    i, k = j % slots, j // slots
                    sync.wait_ge(gsems[i], (k + 1) * group * 16)
                    sync.dma_start(out=y_r[j], in_=dts[i][:]).then_inc(
                        ssems[i], 16
                    )
                for i in range(slots):
                    sync.wait_ge(ssems[i], rounds(i) * 16)
                sync.wait_ge(isem, 16)
                sync.wait_ge(isem2, 16)

            @block.gpsimd
            def _(g_):
                if warm:
                    # Warm the Q7 indirect ucode + the qGpSimdDynamic ring
                    # while the index head load is in flight: gather row 0
                    # into a scratch tile nothing else touches.
                    g_.memset(wi[:], 0)
                    _indirect_dma(g_, wt[:], x[:], wi[:]).then_inc(wsem, 16)
                g_.wait_ge(isem, 16)
                for j in range(n_groups):
                    i, k = j % slots, j // slots
                    if j * group == head:
                        g_.wait_ge(isem2, 16)
                    if j >= slots:
                        g_.wait_ge(ssems[i], k * 16)
                    for g in range(group):
                        c = j * group + g
                        _indirect_dma(
                            g_,
                            dts[i][:, g * D : (g + 1) * D],
                            x[:],
                            it[:, c : c + 1],
                        ).then_inc(gsems[i], 16)
                if warm:
                    g_.wait_ge(wsem, 16)

        # Block exit emitted per-engine drains + a sem-only barrier; all DMA
        # completions were explicitly waited above, so a plain range-clear
        # (no dge drain) suffices to make the NEFF re-executable.
        sem_nums = sorted(
            [isem.num, isem2.num, wsem.num]
            + [s.num for s in gsems]
            + [s.num for s in ssems]
        )
        assert sem_nums == list(range(sem_nums[0], sem_nums[-1] + 1))
        nc.gpsimd.sem_clear(range(sem_nums[0], sem_nums[-1] + 1))

    return nc


def _get_nc():
    global _cached
    if _cached is None:
        _cached = _build_nc()
    return _cached


def _make_in_maps(inputs, perm):
    x = np.ascontiguousarray(np.asarray(inputs, dtype=np.float32))
    p = np.asarray(perm).astype(np.int64)
    inv = np.empty(B, dtype=np.int32)
    inv[p] = np.arange(B, dtype=np.int32)
    maps = []
    for k in range(N_CORES):
        sl = inv[k * R : (k + 1) * R]
        # idxT[p, j*group + g] = inv_k[j*group*128 + p*group + g]
        n_groups = NCH // GROUP
        idxT = (
            sl.reshape(n_groups, P, GROUP).transpose(1, 0, 2).reshape(P, NCH)
        )
        maps.append({"x": x, "idxT": np.ascontiguousarray(idxT)})
    return maps


def kernel(**kw):
    from concourse.bass_utils import run_bass_kernel_spmd

    nc = _get_nc()
    in_maps = _make_in_maps(kw["inputs"], kw["perm"])
    res = run_bass_kernel_spmd(nc, in_maps, core_ids=list(range(N_CORES)))
    return np.concatenate([res.results[k]["y"] for k in range(N_CORES)], axis=0)


def run_traced(inputs, perm, **trace_kw):
    """test.py helper: same as kernel() but returns (out, BassKernelResults)."""
    from concourse.bass_utils import run_bass_kernel_spmd

    nc = _get_nc()
    in_maps = _make_in_maps(inputs, perm)
    res = run_bass_kernel_spmd(
        nc, in_maps, core_ids=list(range(N_CORES)), trace=True, **trace_kw
    )
    out = np.concatenate([res.results[k]["y"] for k in range(N_CORES)], axis=0)
    return out, res
